# revision 1
# baseline (speedup 1.0000x reference)
"""Trainium2 Bass kernel for DoubleBinaryLinear:
    y = ((x * s0) @ B.T * s2) @ A.T * s4 + bias
with x [4, 2048, 4096] fp32 and binary (+-1) B, A [4096, 4096].

Strategy
--------
Data-parallel over tokens: the 8192 tokens are split 1024 per NeuronCore
(8 cores), each core runs the full two-layer pipeline on its token slice.
No collectives. Weights are replicated (B.T / A.T, cast to bf16 — exact
for +-1 entries).

On-chip dataflow keeps activations transposed ([feature_partition,
token_free]) so every diagonal scaling is a per-partition tensor_scalar op
and matmul contractions run over the partition axis:

  stage 0: z = xT * s0, split into bf16 hi + lo   (hi = rne(z), lo = rne(z - hi))
  stage 1: h1 = B @ z     -- PSUM fp32 accumulation of hi and lo passes
  stage 2: h1s = h1 * s2, split into bf16 hi + lo
  stage 3: h2 = A @ h1s   -- PSUM fp32 accumulation
  stage 4: yT = h2 * s4 + bias

The bf16 hi/lo split keeps ~16 mantissa bits of each activation while the
+-1 weights are exact in bf16, so each matmul runs on the 1-cycle/row bf16
path (vs 4 cycles/row for fp32): measured end-to-end error ~3.6e-6 of the
output scale, at ~2x the speed of native-fp32 matmuls.

Measured on trn2 (8 cores): ~1.797 ms HW exec, 99.2% TensorE occupancy
(median matmul issue gap 216 ns = the N=512 bf16 issue bound), PE busy
at the 2-pass issue bound 8192 x 216 ns = 1.766 ms/core. x/y-tile DMAs
ride the Activation HWDGE queue so weight streams own the Sync queue.
"""

import os

import numpy as np
import ml_dtypes

import concourse.bacc as bacc
import concourse.mybir as mybir
from concourse import tile
from concourse import bass_utils

P = 128
F32 = mybir.dt.float32
BF16 = mybir.dt.bfloat16

IN_D = 4096
MID_D = 4096
OUT_D = 4096
BATCH = 4
SEQ = 2048
N_CORES = 8
T_CORE = (BATCH * SEQ) // N_CORES   # 1024 tokens per core
TC = 512                            # token chunk = matmul moving free dim
MG = 4                              # m-tiles per PSUM group


def _build_nc():
    nI, nM, nO, nC = IN_D // P, MID_D // P, OUT_D // P, T_CORE // TC

    nc = bacc.Bacc(None, target_bir_lowering=False)
    xT = nc.dram_tensor("xT", [IN_D, T_CORE], F32, kind="ExternalInput")
    BTd = nc.dram_tensor("BT", [IN_D, MID_D], BF16, kind="ExternalInput")
    ATd = nc.dram_tensor("AT", [MID_D, OUT_D], BF16, kind="ExternalInput")
    nSC = (IN_D + MID_D + 2 * OUT_D) // P
    scd = nc.dram_tensor("sc", [P, nSC], F32, kind="ExternalInput")
    yT = nc.dram_tensor("yT", [OUT_D, T_CORE], F32, kind="ExternalOutput")

    mult = mybir.AluOpType.mult
    sub = mybir.AluOpType.subtract
    add = mybir.AluOpType.add

    with tile.TileContext(nc) as tc:
        with (
            tc.tile_pool(name="consts", bufs=1) as cpool,
            tc.tile_pool(name="zbuf", bufs=1) as zpool,
            tc.tile_pool(name="h1buf", bufs=1) as hpool,
            tc.tile_pool(name="xin", bufs=4) as xpool,
            tc.tile_pool(name="wts", bufs=6) as wpool,
            tc.tile_pool(name="yout", bufs=3) as ypool,
            tc.tile_pool(name="psum", bufs=8, space="PSUM") as pspool,
        ):
            sc_t = cpool.tile([P, nSC], F32, tag="sc")
            nc.sync.dma_start(sc_t[:], scd[:, :])
            s0_t = sc_t[:, 0:nI]
            s2_t = sc_t[:, nI:nI + nM]
            s4_t = sc_t[:, nI + nM:nI + nM + nO]
            bi_t = sc_t[:, nI + nM + nO:nSC]

            for c in range(nC):
                t0 = c * TC
                # stage 0: load x, scale by s0, split to bf16 hi/lo
                zhi = [zpool.tile([P, TC], BF16, tag=f"zhi{i}", name=f"zhi{i}")
                       for i in range(nI)]
                zlo = [zpool.tile([P, TC], BF16, tag=f"zlo{i}", name=f"zlo{i}")
                       for i in range(nI)]
                for i in range(nI):
                    xt = xpool.tile([P, TC], F32, tag="xt")
                    nc.scalar.dma_start(xt[:], xT[i * P:(i + 1) * P, t0:t0 + TC])
                    if c == 0 and i < 2:
                        nc.vector.tensor_scalar_mul(
                            zhi[i][:], xt[:], s0_t[:, i:i + 1])
                    else:
                        nc.scalar.activation(zhi[i][:], xt[:],
                                             mybir.ActivationFunctionType.Copy,
                                             scale=s0_t[:, i:i + 1])
                    nc.vector.scalar_tensor_tensor(
                        zlo[i][:], xt[:], s0_t[:, i:i + 1], zhi[i][:], mult, sub)

                # stage 1: h1 = B @ z; stage 2: scale by s2, split
                h1hi = [hpool.tile([P, TC], BF16, tag=f"h1hi{m}", name=f"h1hi{m}")
                        for m in range(nM)]
                h1lo = [hpool.tile([P, TC], BF16, tag=f"h1lo{m}", name=f"h1lo{m}")
                        for m in range(nM)]
                for mg in range(nM // MG):
                    pss = [pspool.tile([P, TC], F32, tag="ps", name="ps")
                           for _ in range(MG)]
                    for i in range(nI):
                        wt = wpool.tile([P, MG * P], BF16, tag="wb")
                        nc.sync.dma_start(
                            wt[:], BTd[i * P:(i + 1) * P,
                                       mg * MG * P:(mg + 1) * MG * P])
                        last_i = i == nI - 1
                        for m_ in range(MG):
                            lhsT = wt[:, m_ * P:(m_ + 1) * P]
                            nc.tensor.matmul(pss[m_][:], lhsT, zhi[i][:],
                                             start=(i == 0), stop=False)
                            nc.tensor.matmul(pss[m_][:], lhsT, zlo[i][:],
                                             start=False, stop=last_i)
                    for m_ in range(MG):
                        m = mg * MG + m_
                        nc.scalar.activation(
                            h1hi[m][:], pss[m_][:],
                            mybir.ActivationFunctionType.Copy,
                            scale=s2_t[:, m:m + 1])
                        nc.vector.scalar_tensor_tensor(
                            h1lo[m][:], pss[m_][:], s2_t[:, m:m + 1],
                            h1hi[m][:], mult, sub)

                # stage 3: h2 = A @ h1s; stage 4: y = h2*s4 + bias
                for og in range(nO // MG):
                    pso = [pspool.tile([P, TC], F32, tag="ps", name="ps")
                           for _ in range(MG)]
                    for m in range(nM):
                        wt2 = wpool.tile([P, MG * P], BF16, tag="wa")
                        nc.sync.dma_start(
                            wt2[:], ATd[m * P:(m + 1) * P,
                                        og * MG * P:(og + 1) * MG * P])
                        last_m = m == nM - 1
                        for o_ in range(MG):
                            lhsT = wt2[:, o_ * P:(o_ + 1) * P]
                            nc.tensor.matmul(pso[o_][:], lhsT, h1hi[m][:],
                                             start=(m == 0), stop=False)
                            nc.tensor.matmul(pso[o_][:], lhsT, h1lo[m][:],
                                             start=False, stop=last_m)
                    for o_ in range(MG):
                        o = og * MG + o_
                        yt = ypool.tile([P, TC], F32, tag="yt")
                        nc.vector.tensor_scalar(
                            yt[:], pso[o_][:], s4_t[:, o:o + 1], bi_t[:, o:o + 1],
                            mult, add)
                        nc.scalar.dma_start(yT[o * P:(o + 1) * P, t0:t0 + TC], yt[:])

    nc.compile()
    return nc


_NC_CACHE = None


def _get_nc():
    global _NC_CACHE
    if _NC_CACHE is None:
        _NC_CACHE = _build_nc()
    return _NC_CACHE


def _col_major(v):
    return np.ascontiguousarray(
        np.asarray(v, dtype=np.float32).reshape(-1, P).T)


def make_in_maps(x, scaling0, B, scaling2, A, scaling4, bias):
    x = np.asarray(x, dtype=np.float32)
    xf = np.ascontiguousarray(x.reshape(BATCH * SEQ, IN_D))
    BT = np.ascontiguousarray(np.asarray(B, dtype=np.float32).T
                              ).astype(ml_dtypes.bfloat16)
    AT = np.ascontiguousarray(np.asarray(A, dtype=np.float32).T
                              ).astype(ml_dtypes.bfloat16)
    sc = np.ascontiguousarray(np.concatenate(
        [_col_major(v) for v in (scaling0, scaling2, scaling4, bias)], axis=1))

    in_maps = []
    for c in range(N_CORES):
        xs = xf[c * T_CORE:(c + 1) * T_CORE]
        in_maps.append({
            "xT": np.ascontiguousarray(xs.T),
            "BT": BT, "AT": AT, "sc": sc,
        })
    return in_maps


def kernel(x, scaling0, B, scaling2, A, scaling4, bias):
    # The profile hook isn't available in every environment; force the
    # plain execution path.
    os.environ.setdefault("BASS_NEVER_TRACE", "1")

    in_maps = make_in_maps(x, scaling0, B, scaling2, A, scaling4, bias)
    nc = _get_nc()
    res = bass_utils.run_bass_kernel_spmd(
        nc, in_maps, core_ids=list(range(N_CORES)))

    y = np.empty((BATCH * SEQ, OUT_D), dtype=np.float32)
    for c in range(N_CORES):
        y[c * T_CORE:(c + 1) * T_CORE] = res.results[c]["yT"].T
    return y.reshape(BATCH, SEQ, OUT_D)



# revision 2
# speedup vs baseline: 1.3099x; 1.3099x over previous
"""Trainium2 Bass kernel for DoubleBinaryLinear:
    y = ((x * s0) @ B.T * s2) @ A.T * s4 + bias
with x [4, 2048, 4096] fp32 and binary (+-1) B, A [4096, 4096].

v2: data-parallel over tokens (1024/core), single-pass fp16 matmuls.
The +-1 weights are exact in fp16; only activations are rounded once per
layer (rel err ~1e-3 << 2e-2 gate), so each 128x512 matmul runs once
instead of the baseline's hi/lo pair -- 2x less TensorE work.
"""

import os

import numpy as np
import ml_dtypes

import concourse.bacc as bacc
import concourse.mybir as mybir
from concourse import tile
from concourse import bass_utils

P = 128
F32 = mybir.dt.float32
F16 = mybir.dt.float16

IN_D = 4096
MID_D = 4096
OUT_D = 4096
BATCH = 4
SEQ = 2048
N_CORES = 8
T_CORE = (BATCH * SEQ) // N_CORES   # 1024 tokens per core
TC = 512                            # matmul moving free dim
MG = 4                              # m-tiles per PSUM group


def _build_nc():
    nI, nM, nO, nC = IN_D // P, MID_D // P, OUT_D // P, T_CORE // TC

    nc = bacc.Bacc(None, target_bir_lowering=False)
    xT = nc.dram_tensor("xT", [IN_D, T_CORE], F32, kind="ExternalInput")
    BTd = nc.dram_tensor("BT", [IN_D, MID_D], F16, kind="ExternalInput")
    ATd = nc.dram_tensor("AT", [MID_D, OUT_D], F16, kind="ExternalInput")
    nSC = (IN_D + MID_D + 2 * OUT_D) // P
    scd = nc.dram_tensor("sc", [P, nSC], F32, kind="ExternalInput")
    yT = nc.dram_tensor("yT", [OUT_D, T_CORE], F32, kind="ExternalOutput")

    mult = mybir.AluOpType.mult
    add = mybir.AluOpType.add

    with tile.TileContext(nc) as tc:
        with (
            tc.tile_pool(name="consts", bufs=1) as cpool,
            tc.tile_pool(name="zbuf", bufs=1) as zpool,
            tc.tile_pool(name="h1buf", bufs=1) as hpool,
            tc.tile_pool(name="xin", bufs=4) as xpool,
            tc.tile_pool(name="wts", bufs=6) as wpool,
            tc.tile_pool(name="yout", bufs=3) as ypool,
            tc.tile_pool(name="psum", bufs=8, space="PSUM") as pspool,
        ):
            sc_t = cpool.tile([P, nSC], F32, tag="sc")
            nc.sync.dma_start(sc_t[:], scd[:, :])
            s0_t = sc_t[:, 0:nI]
            s2_t = sc_t[:, nI:nI + nM]
            s4_t = sc_t[:, nI + nM:nI + nM + nO]
            bi_t = sc_t[:, nI + nM + nO:nSC]

            for c in range(nC):
                t0 = c * TC
                # stage 0: load x, scale by s0, round to bf16
                z = [zpool.tile([P, TC], F16, tag=f"z{i}", name=f"z{i}")
                     for i in range(nI)]
                for i in range(nI):
                    xt = xpool.tile([P, TC], F32, tag="xt")
                    nc.scalar.dma_start(xt[:], xT[i * P:(i + 1) * P, t0:t0 + TC])
                    if c == 0 and i < 2:
                        nc.vector.tensor_scalar_mul(
                            z[i][:], xt[:], s0_t[:, i:i + 1])
                    else:
                        nc.scalar.activation(z[i][:], xt[:],
                                             mybir.ActivationFunctionType.Copy,
                                             scale=s0_t[:, i:i + 1])

                # stage 1: h1 = B @ z; stage 2: scale by s2, round to bf16
                h1 = [hpool.tile([P, TC], F16, tag=f"h1{m}", name=f"h1{m}")
                      for m in range(nM)]
                for mg in range(nM // MG):
                    pss = [pspool.tile([P, TC], F32, tag="ps", name="ps")
                           for _ in range(MG)]
                    for i in range(nI):
                        wt = wpool.tile([P, MG * P], F16, tag="wb")
                        nc.sync.dma_start(
                            wt[:], BTd[i * P:(i + 1) * P,
                                       mg * MG * P:(mg + 1) * MG * P])
                        last_i = i == nI - 1
                        for m_ in range(MG):
                            lhsT = wt[:, m_ * P:(m_ + 1) * P]
                            nc.tensor.matmul(pss[m_][:], lhsT, z[i][:],
                                             start=(i == 0), stop=last_i)
                    for m_ in range(MG):
                        m = mg * MG + m_
                        nc.scalar.activation(
                            h1[m][:], pss[m_][:],
                            mybir.ActivationFunctionType.Copy,
                            scale=s2_t[:, m:m + 1])

                # stage 3: h2 = A @ h1s; stage 4: y = h2*s4 + bias
                for og in range(nO // MG):
                    pso = [pspool.tile([P, TC], F32, tag="ps", name="ps")
                           for _ in range(MG)]
                    for m in range(nM):
                        wt2 = wpool.tile([P, MG * P], F16, tag="wa")
                        nc.sync.dma_start(
                            wt2[:], ATd[m * P:(m + 1) * P,
                                        og * MG * P:(og + 1) * MG * P])
                        last_m = m == nM - 1
                        for o_ in range(MG):
                            lhsT = wt2[:, o_ * P:(o_ + 1) * P]
                            nc.tensor.matmul(pso[o_][:], lhsT, h1[m][:],
                                             start=(m == 0), stop=last_m)
                    for o_ in range(MG):
                        o = og * MG + o_
                        yt = ypool.tile([P, TC], F32, tag="yt")
                        nc.vector.tensor_scalar(
                            yt[:], pso[o_][:], s4_t[:, o:o + 1], bi_t[:, o:o + 1],
                            mult, add)
                        nc.scalar.dma_start(yT[o * P:(o + 1) * P, t0:t0 + TC], yt[:])

    nc.compile()
    return nc


_NC_CACHE = None


def _get_nc():
    global _NC_CACHE
    if _NC_CACHE is None:
        _NC_CACHE = _build_nc()
    return _NC_CACHE


def _col_major(v):
    return np.ascontiguousarray(
        np.asarray(v, dtype=np.float32).reshape(-1, P).T)


def make_in_maps(x, scaling0, B, scaling2, A, scaling4, bias):
    x = np.asarray(x, dtype=np.float32)
    xf = np.ascontiguousarray(x.reshape(BATCH * SEQ, IN_D))
    BT = np.ascontiguousarray(np.asarray(B, dtype=np.float32).T
                              ).astype(np.float16)
    AT = np.ascontiguousarray(np.asarray(A, dtype=np.float32).T
                              ).astype(np.float16)
    sc = np.ascontiguousarray(np.concatenate(
        [_col_major(v) for v in (scaling0, scaling2, scaling4, bias)], axis=1))

    in_maps = []
    for c in range(N_CORES):
        xs = xf[c * T_CORE:(c + 1) * T_CORE]
        in_maps.append({
            "xT": np.ascontiguousarray(xs.T),
            "BT": BT, "AT": AT, "sc": sc,
        })
    return in_maps


def kernel(x, scaling0, B, scaling2, A, scaling4, bias):
    # The profile hook isn't available in every environment; force the
    # plain execution path.
    os.environ.setdefault("BASS_NEVER_TRACE", "1")

    in_maps = make_in_maps(x, scaling0, B, scaling2, A, scaling4, bias)
    nc = _get_nc()
    res = bass_utils.run_bass_kernel_spmd(
        nc, in_maps, core_ids=list(range(N_CORES)))

    y = np.empty((BATCH * SEQ, OUT_D), dtype=np.float32)
    for c in range(N_CORES):
        y[c * T_CORE:(c + 1) * T_CORE] = res.results[c]["yT"].T
    return y.reshape(BATCH, SEQ, OUT_D)


# revision 3
# speedup vs baseline: 1.3412x; 1.0239x over previous
"""Trainium2 Bass kernel for DoubleBinaryLinear:
    y = ((x * s0) @ B.T * s2) @ A.T * s4 + bias
with x [4, 2048, 4096] fp32 and binary (+-1) B, A [4096, 4096].

v5: fused-weight restructure, output-sharded across the 8 cores.

    M.T = diag(s0) B.T (diag(s2) A.T)   # [in, out], token-independent
    y   = x @ M.T * s4 + bias           # x pre-cast fp16 on host

Core c computes M.T columns for its 512 output rows (one 4096x4096x512
matmul, ~220 us) and keeps M.T resident in SBUF, then streams ALL 8192
tokens through a single fused matmul (~440 us). Out-sharding makes each
core's M shard exactly what its own x-pass needs -- no collective, and
the token loop has no per-tile activations (s0 rides the M eviction
scale). All matmuls single-pass fp16 (binary weights exact; M and x
rounded once, rel err ~4e-4 << 2e-2 gate).

Per-matmul operands must come from plain [128, 512] tiles: slicing a
wider (batched-DMA) tile gives the operand AP a partition stride larger
than its free extent, which drops TensorE off its fast path (measured
259 ns vs 216 ns per 512-row matmul). So DMAs here are one trigger per
[128, 512] tile; evictions run on the Vector engine so the Scalar
engine is a pure x-prefetch stream and Sync carries B/A/y.
"""

import os

import numpy as np

import concourse.bacc as bacc
import concourse.mybir as mybir
from concourse import tile
from concourse import bass_utils

P = 128
F32 = mybir.dt.float32
F16 = mybir.dt.float16

IN_D = 4096
MID_D = 4096
OUT_D = 4096
BATCH = 4
SEQ = 2048
N_CORES = 8
T_ALL = BATCH * SEQ                 # 8192 tokens, every core sees all
OS = OUT_D // N_CORES               # 512 output rows per core
TC = 512                            # matmul moving free dim
nI = IN_D // P                      # 32 in tiles
nM = MID_D // P                     # 32 mid tiles
nOB = OS // P                       # 4 out blocks per core
nTC = T_ALL // TC                   # 16 token chunks
IG = 4                              # in-tiles per M-compute PSUM group


def _build_nc():
    nc = bacc.Bacc(None, target_bir_lowering=False)
    xTd = nc.dram_tensor("xT", [IN_D, T_ALL], F16, kind="ExternalInput")
    Bd = nc.dram_tensor("B", [MID_D, IN_D], F16, kind="ExternalInput")
    ATd = nc.dram_tensor("ATs", [MID_D, OS], F16, kind="ExternalInput")
    nSC = nI + nM + 2 * nOB
    scd = nc.dram_tensor("sc", [P, nSC], F32, kind="ExternalInput")
    yTd = nc.dram_tensor("yT", [OS, T_ALL], F32, kind="ExternalOutput")

    mult = mybir.AluOpType.mult
    add = mybir.AluOpType.add

    with tile.TileContext(nc) as tc:
        with (
            tc.tile_pool(name="consts", bufs=1) as cpool,
            tc.tile_pool(name="a2buf", bufs=1) as apool,
            tc.tile_pool(name="mtbuf", bufs=1) as mpool,
            tc.tile_pool(name="xin", bufs=2) as xpool,
            tc.tile_pool(name="bwts", bufs=8) as bpool,
            tc.tile_pool(name="stage", bufs=4) as spool,
            tc.tile_pool(name="yout", bufs=6) as ypool,
            tc.tile_pool(name="psum", bufs=8, space="PSUM") as pspool,
        ):
            sc_t = cpool.tile([P, nSC], F32, tag="sc")
            nc.sync.dma_start(sc_t[:], scd[:, :])
            s0_t = sc_t[:, 0:nI]
            s2_t = sc_t[:, nI:nI + nM]
            s4_t = sc_t[:, nI + nM:nI + nM + nOB]
            bi_t = sc_t[:, nI + nM + nOB:nSC]

            # phase A: a2[mk] = fp16(s2 * A.T) tiles [128 mid, OS]
            a2 = []
            for mk in range(nM):
                at = spool.tile([P, OS], F16, tag="at")
                nc.sync.dma_start(at[:], ATd[mk * P:(mk + 1) * P, :])
                a2t = apool.tile([P, OS], F16, tag=f"a{mk}", name=f"a{mk}")
                nc.vector.tensor_scalar_mul(a2t[:], at[:], s2_t[:, mk:mk + 1])
                a2.append(a2t)

            # phase B: M.T tiles [128 in, OS]; s0 folded into eviction
            mt = [mpool.tile([P, OS], F16, tag=f"m{it}", name=f"m{it}")
                  for it in range(nI)]
            for ig in range(nI // IG):
                psb = [pspool.tile([P, OS], F32, tag="ps", name="ps")
                       for _ in range(IG)]
                for mk in range(nM):
                    bt = bpool.tile([P, IG * P], F16, tag="wb")
                    nc.sync.dma_start(
                        bt[:], Bd[mk * P:(mk + 1) * P,
                                  ig * IG * P:(ig + 1) * IG * P])
                    last = mk == nM - 1
                    for t_ in range(IG):
                        nc.tensor.matmul(psb[t_][:], bt[:, t_ * P:(t_ + 1) * P],
                                         a2[mk][:], start=(mk == 0), stop=last)
                for t_ in range(IG):
                    it = ig * IG + t_
                    nc.vector.tensor_scalar_mul(mt[it][:], psb[t_][:],
                                                s0_t[:, it:it + 1])

            # phase C: stream tokens; y = x @ M.T * s4 + bias
            for c in range(nTC):
                t0 = c * TC
                xts = []
                for it in range(nI):
                    xt = xpool.tile([P, TC], F16, tag=f"x{it}", name=f"x{it}")
                    nc.scalar.dma_start(
                        xt[:], xTd[it * P:(it + 1) * P, t0:t0 + TC])
                    xts.append(xt)
                pso = [pspool.tile([P, TC], F32, tag="ps", name="ps")
                       for _ in range(nOB)]
                for it in range(nI):
                    for ob in range(nOB):
                        nc.tensor.matmul(pso[ob][:],
                                         mt[it][:, ob * P:(ob + 1) * P],
                                         xts[it][:], start=(it == 0),
                                         stop=(it == nI - 1))
                for ob in range(nOB):
                    yt = ypool.tile([P, TC], F32, tag="yt")
                    nc.vector.tensor_scalar(
                        yt[:], pso[ob][:], s4_t[:, ob:ob + 1], bi_t[:, ob:ob + 1],
                        mult, add)
                    nc.sync.dma_start(yTd[ob * P:(ob + 1) * P, t0:t0 + TC], yt[:])

    nc.compile()
    return nc


_NC_CACHE = None


def _get_nc():
    global _NC_CACHE
    if _NC_CACHE is None:
        _NC_CACHE = _build_nc()
    return _NC_CACHE


def _col_major(v):
    return np.ascontiguousarray(
        np.asarray(v, dtype=np.float32).reshape(-1, P).T)


def make_in_maps(x, scaling0, B, scaling2, A, scaling4, bias):
    xh = np.asarray(x, dtype=np.float32).reshape(T_ALL, IN_D).astype(np.float16)
    xT = np.ascontiguousarray(xh.T)
    B16 = np.asarray(B, dtype=np.float32).astype(np.float16)
    AT = np.asarray(A, dtype=np.float32).T.astype(np.float16)
    s0c = _col_major(scaling0)
    s2c = _col_major(scaling2)

    in_maps = []
    for c in range(N_CORES):
        sh = slice(c * OS, (c + 1) * OS)
        sc = np.ascontiguousarray(np.concatenate(
            [s0c, s2c, _col_major(np.asarray(scaling4)[sh]),
             _col_major(np.asarray(bias)[sh])], axis=1))
        in_maps.append({
            "xT": xT, "B": B16,
            "ATs": np.ascontiguousarray(AT[:, sh]),
            "sc": sc,
        })
    return in_maps


def _unshard(results):
    y = np.empty((T_ALL, OUT_D), dtype=np.float32)
    for c in range(N_CORES):
        y[:, c * OS:(c + 1) * OS] = results[c]["yT"].T
    return y.reshape(BATCH, SEQ, OUT_D)


def kernel(x, scaling0, B, scaling2, A, scaling4, bias):
    # The profile hook isn't available in every environment; force the
    # plain execution path.
    os.environ.setdefault("BASS_NEVER_TRACE", "1")

    in_maps = make_in_maps(x, scaling0, B, scaling2, A, scaling4, bias)
    nc = _get_nc()
    res = bass_utils.run_bass_kernel_spmd(
        nc, in_maps, core_ids=list(range(N_CORES)))
    return _unshard(res.results)


# revision 4
# speedup vs baseline: 1.3550x; 1.0103x over previous
"""Trainium2 Bass kernel for DoubleBinaryLinear:
    y = ((x * s0) @ B.T * s2) @ A.T * s4 + bias
with x [4, 2048, 4096] fp32 and binary (+-1) B, A [4096, 4096].

v5: fused-weight restructure, output-sharded across the 8 cores.

    M.T = diag(s0) B.T (diag(s2) A.T)   # [in, out], token-independent
    y   = x @ M.T * s4 + bias           # x pre-cast fp16 on host

Core c computes M.T columns for its 512 output rows (one 4096x4096x512
matmul, ~220 us) and keeps M.T resident in SBUF, then streams ALL 8192
tokens through a single fused matmul (~440 us). Out-sharding makes each
core's M shard exactly what its own x-pass needs -- no collective, and
the token loop has no per-tile activations (s0 rides the M eviction
scale). All matmuls single-pass fp16 (binary weights exact; M and x
rounded once, rel err ~4e-4 << 2e-2 gate).

Per-matmul operands must come from plain [128, 512] tiles: slicing a
wider (batched-DMA) tile gives the operand AP a partition stride larger
than its free extent, which drops TensorE off its fast path (measured
259 ns vs 216 ns per 512-row matmul). So DMAs here are one trigger per
[128, 512] tile; evictions run on the Vector engine so the Scalar
engine is a pure x-prefetch stream and Sync carries B/A/y.
"""

import os

import numpy as np

import concourse.bacc as bacc
import concourse.mybir as mybir
from concourse import tile
from concourse import bass_utils

P = 128
F32 = mybir.dt.float32
F16 = mybir.dt.float16

IN_D = 4096
MID_D = 4096
OUT_D = 4096
BATCH = 4
SEQ = 2048
N_CORES = 8
T_ALL = BATCH * SEQ                 # 8192 tokens, every core sees all
OS = OUT_D // N_CORES               # 512 output rows per core
TC = 512                            # matmul moving free dim
nI = IN_D // P                      # 32 in tiles
nM = MID_D // P                     # 32 mid tiles
nOB = OS // P                       # 4 out blocks per core
nTC = T_ALL // TC                   # 16 token chunks
IG = 4                              # in-tiles per M-compute PSUM group


def _build_nc():
    nc = bacc.Bacc(None, target_bir_lowering=False)
    xTd = nc.dram_tensor("xT", [IN_D, T_ALL], F16, kind="ExternalInput")
    Bd = nc.dram_tensor("B", [MID_D, IN_D], F16, kind="ExternalInput")
    ATd = nc.dram_tensor("ATs", [MID_D, OS], F16, kind="ExternalInput")
    nSC = nI + nM + 2 * nOB
    scd = nc.dram_tensor("sc", [P, nSC], F32, kind="ExternalInput")
    yTd = nc.dram_tensor("yT", [OS, T_ALL], F32, kind="ExternalOutput")

    mult = mybir.AluOpType.mult
    add = mybir.AluOpType.add

    with tile.TileContext(nc) as tc:
        with (
            tc.tile_pool(name="consts", bufs=1) as cpool,
            tc.tile_pool(name="a2buf", bufs=1) as apool,
            tc.tile_pool(name="mtbuf", bufs=1) as mpool,
            tc.tile_pool(name="xin", bufs=2) as xpool,
            tc.tile_pool(name="bwts", bufs=12) as bpool,
            tc.tile_pool(name="stage", bufs=4) as spool,
            tc.tile_pool(name="yout", bufs=6) as ypool,
            tc.tile_pool(name="psum", bufs=8, space="PSUM") as pspool,
        ):
            sc_t = cpool.tile([P, nSC], F32, tag="sc")
            nc.scalar.dma_start(sc_t[:], scd[:, :])
            s0_t = sc_t[:, 0:nI]
            s2_t = sc_t[:, nI:nI + nM]
            s4_t = sc_t[:, nI + nM:nI + nM + nOB]
            bi_t = sc_t[:, nI + nM + nOB:nSC]

            # phase A: a2[mk] = fp16(s2 * A.T) tiles [128 mid, OS]
            a2 = []
            for mk in range(nM):
                at = spool.tile([P, OS], F16, tag="at")
                nc.scalar.dma_start(at[:], ATd[mk * P:(mk + 1) * P, :])
                a2t = apool.tile([P, OS], F16, tag=f"a{mk}", name=f"a{mk}")
                nc.vector.tensor_scalar_mul(a2t[:], at[:], s2_t[:, mk:mk + 1])
                a2.append(a2t)

            # phase B: M.T tiles [128 in, OS]; s0 folded into eviction
            mt = [mpool.tile([P, OS], F16, tag=f"m{it}", name=f"m{it}")
                  for it in range(nI)]
            for ig in range(nI // IG):
                psb = [pspool.tile([P, OS], F32, tag="ps", name="ps")
                       for _ in range(IG)]
                for mk in range(nM):
                    bt = bpool.tile([P, IG * P], F16, tag="wb")
                    nc.sync.dma_start(
                        bt[:], Bd[mk * P:(mk + 1) * P,
                                  ig * IG * P:(ig + 1) * IG * P])
                    last = mk == nM - 1
                    for t_ in range(IG):
                        nc.tensor.matmul(psb[t_][:], bt[:, t_ * P:(t_ + 1) * P],
                                         a2[mk][:], start=(mk == 0), stop=last)
                for t_ in range(IG):
                    it = ig * IG + t_
                    nc.vector.tensor_scalar_mul(mt[it][:], psb[t_][:],
                                                s0_t[:, it:it + 1])

            # phase C: stream tokens; y = x @ M.T * s4 + bias
            for c in range(nTC):
                t0 = c * TC
                xts = []
                for it in range(nI):
                    xt = xpool.tile([P, TC], F16, tag=f"x{it}", name=f"x{it}")
                    nc.scalar.dma_start(
                        xt[:], xTd[it * P:(it + 1) * P, t0:t0 + TC])
                    xts.append(xt)
                pso = [pspool.tile([P, TC], F32, tag="ps", name="ps")
                       for _ in range(nOB)]
                for it in range(nI):
                    for ob in range(nOB):
                        nc.tensor.matmul(pso[ob][:],
                                         mt[it][:, ob * P:(ob + 1) * P],
                                         xts[it][:], start=(it == 0),
                                         stop=(it == nI - 1))
                for ob in range(nOB):
                    yt = ypool.tile([P, TC], F32, tag="yt")
                    nc.vector.tensor_scalar(
                        yt[:], pso[ob][:], s4_t[:, ob:ob + 1], bi_t[:, ob:ob + 1],
                        mult, add)
                    nc.sync.dma_start(yTd[ob * P:(ob + 1) * P, t0:t0 + TC], yt[:])

    nc.compile()
    return nc


_NC_CACHE = None


def _get_nc():
    global _NC_CACHE
    if _NC_CACHE is None:
        _NC_CACHE = _build_nc()
    return _NC_CACHE


def _col_major(v):
    return np.ascontiguousarray(
        np.asarray(v, dtype=np.float32).reshape(-1, P).T)


def make_in_maps(x, scaling0, B, scaling2, A, scaling4, bias):
    xh = np.asarray(x, dtype=np.float32).reshape(T_ALL, IN_D).astype(np.float16)
    xT = np.ascontiguousarray(xh.T)
    B16 = np.asarray(B, dtype=np.float32).astype(np.float16)
    AT = np.asarray(A, dtype=np.float32).T.astype(np.float16)
    s0c = _col_major(scaling0)
    s2c = _col_major(scaling2)

    in_maps = []
    for c in range(N_CORES):
        sh = slice(c * OS, (c + 1) * OS)
        sc = np.ascontiguousarray(np.concatenate(
            [s0c, s2c, _col_major(np.asarray(scaling4)[sh]),
             _col_major(np.asarray(bias)[sh])], axis=1))
        in_maps.append({
            "xT": xT, "B": B16,
            "ATs": np.ascontiguousarray(AT[:, sh]),
            "sc": sc,
        })
    return in_maps


def _unshard(results):
    y = np.empty((T_ALL, OUT_D), dtype=np.float32)
    for c in range(N_CORES):
        y[:, c * OS:(c + 1) * OS] = results[c]["yT"].T
    return y.reshape(BATCH, SEQ, OUT_D)


def kernel(x, scaling0, B, scaling2, A, scaling4, bias):
    # The profile hook isn't available in every environment; force the
    # plain execution path.
    os.environ.setdefault("BASS_NEVER_TRACE", "1")

    in_maps = make_in_maps(x, scaling0, B, scaling2, A, scaling4, bias)
    nc = _get_nc()
    res = bass_utils.run_bass_kernel_spmd(
        nc, in_maps, core_ids=list(range(N_CORES)))
    return _unshard(res.results)


# revision 5
# speedup vs baseline: 1.3607x; 1.0042x over previous
"""Trainium2 Bass kernel for DoubleBinaryLinear:
    y = ((x * s0) @ B.T * s2) @ A.T * s4 + bias
with x [4, 2048, 4096] fp32 and binary (+-1) B, A [4096, 4096].

v5: fused-weight restructure, output-sharded across the 8 cores.

    M.T = diag(s0) B.T (diag(s2) A.T)   # [in, out], token-independent
    y   = x @ M.T * s4 + bias           # x pre-cast fp16 on host

Core c computes M.T columns for its 512 output rows (one 4096x4096x512
matmul, ~220 us) and keeps M.T resident in SBUF, then streams ALL 8192
tokens through a single fused matmul (~440 us). Out-sharding makes each
core's M shard exactly what its own x-pass needs -- no collective, and
the token loop has no per-tile activations (s0 rides the M eviction
scale). All matmuls single-pass fp16 (binary weights exact; M and x
rounded once, rel err ~4e-4 << 2e-2 gate).

Per-matmul operands must come from plain [128, 512] tiles: slicing a
wider (batched-DMA) tile gives the operand AP a partition stride larger
than its free extent, which drops TensorE off its fast path (measured
259 ns vs 216 ns per 512-row matmul). So DMAs here are one trigger per
[128, 512] tile; evictions run on the Vector engine so the Scalar
engine is a pure x-prefetch stream and Sync carries B/A/y.
"""

import os

import numpy as np

import concourse.bacc as bacc
import concourse.mybir as mybir
from concourse import tile
from concourse import bass_utils

P = 128
F32 = mybir.dt.float32
F16 = mybir.dt.float16

IN_D = 4096
MID_D = 4096
OUT_D = 4096
BATCH = 4
SEQ = 2048
N_CORES = 8
T_ALL = BATCH * SEQ                 # 8192 tokens, every core sees all
OS = OUT_D // N_CORES               # 512 output rows per core
TC = 512                            # matmul moving free dim
nI = IN_D // P                      # 32 in tiles
nM = MID_D // P                     # 32 mid tiles
nOB = OS // P                       # 4 out blocks per core
nTC = T_ALL // TC                   # 16 token chunks
IG = 4                              # in-tiles per M-compute PSUM group


def _build_nc():
    nc = bacc.Bacc(None, target_bir_lowering=False)
    xTd = nc.dram_tensor("xT", [IN_D, T_ALL], F16, kind="ExternalInput")
    Bd = nc.dram_tensor("B", [MID_D, IN_D], F16, kind="ExternalInput")
    ATd = nc.dram_tensor("ATs", [MID_D, OS], F16, kind="ExternalInput")
    nSC = nI + nM + 2 * nOB
    scd = nc.dram_tensor("sc", [P, nSC], F32, kind="ExternalInput")
    yTd = nc.dram_tensor("yT", [OS, T_ALL], F32, kind="ExternalOutput")

    mult = mybir.AluOpType.mult
    add = mybir.AluOpType.add

    with tile.TileContext(nc) as tc:
        with (
            tc.tile_pool(name="consts", bufs=1) as cpool,
            tc.tile_pool(name="a2buf", bufs=1) as apool,
            tc.tile_pool(name="mtbuf", bufs=1) as mpool,
            tc.tile_pool(name="xin", bufs=2) as xpool,
            tc.tile_pool(name="bwts", bufs=12) as bpool,
            tc.tile_pool(name="stage", bufs=4) as spool,
            tc.tile_pool(name="yout", bufs=6) as ypool,
            tc.tile_pool(name="psum", bufs=8, space="PSUM") as pspool,
        ):
            sc_t = cpool.tile([P, nSC], F32, tag="sc")
            nc.scalar.dma_start(sc_t[:], scd[:, :])
            s0_t = sc_t[:, 0:nI]
            s2_t = sc_t[:, nI:nI + nM]
            s4_t = sc_t[:, nI + nM:nI + nM + nOB]
            bi_t = sc_t[:, nI + nM + nOB:nSC]

            # phase A: a2[mk] = fp16(s2 * A.T) tiles [128 mid, OS]
            a2 = []
            for mk in range(nM):
                at = spool.tile([P, OS], F16, tag="at")
                nc.scalar.dma_start(at[:], ATd[mk * P:(mk + 1) * P, :])
                a2t = apool.tile([P, OS], F16, tag=f"a{mk}", name=f"a{mk}")
                nc.vector.tensor_scalar_mul(a2t[:], at[:], s2_t[:, mk:mk + 1])
                a2.append(a2t)

            # Chunk-0 x tiles prefetch right behind the AT loads so the
            # interleaved chunk-0 matmuls below have data by ~40 us.
            x0 = []
            for it in range(nI):
                xt = xpool.tile([P, TC], F16, tag=f"x{it}", name=f"x{it}")
                nc.scalar.dma_start(xt[:], xTd[it * P:(it + 1) * P, 0:TC])
                x0.append(xt)
            # Chunk-0 accumulators held across all of phase B (4 banks);
            # phase B itself rings through the other 4.
            ps0 = [pspool.tile([P, TC], F32, tag="ps0", name="ps0", bufs=4)
                   for _ in range(nOB)]

            def x0_mm(j, ig):
                # j-th (0..15) interleave slot while phase-B group ig runs:
                # chunk-0 matmul for an in-tile of group ig-1.
                it = (ig - 1) * IG + j // nOB
                ob = j % nOB
                nc.tensor.matmul(ps0[ob][:], mt[it][:, ob * P:(ob + 1) * P],
                                 x0[it][:], start=(it == 0),
                                 stop=(it == nI - 1))

            # phase B: M.T tiles [128 in, OS]; s0 folded into eviction.
            # One chunk-0 matmul per two B-steps stretches the B-stream
            # demand timeline ~11% so DMA jitter stops stalling TensorE.
            mt = [mpool.tile([P, OS], F16, tag=f"m{it}", name=f"m{it}")
                  for it in range(nI)]
            for ig in range(nI // IG):
                psb = [pspool.tile([P, OS], F32, tag="ps", name="ps", bufs=4)
                       for _ in range(IG)]
                for mk in range(nM):
                    bt = bpool.tile([P, IG * P], F16, tag="wb")
                    nc.sync.dma_start(
                        bt[:], Bd[mk * P:(mk + 1) * P,
                                  ig * IG * P:(ig + 1) * IG * P])
                    last = mk == nM - 1
                    for t_ in range(IG):
                        nc.tensor.matmul(psb[t_][:], bt[:, t_ * P:(t_ + 1) * P],
                                         a2[mk][:], start=(mk == 0), stop=last)
                    if ig > 0 and mk % 2 == 1:
                        x0_mm(mk // 2, ig)
                for t_ in range(IG):
                    it = ig * IG + t_
                    nc.vector.tensor_scalar_mul(mt[it][:], psb[t_][:],
                                                s0_t[:, it:it + 1])
            for j in range(16):
                x0_mm(j, nI // IG)
            for ob in range(nOB):
                yt = ypool.tile([P, TC], F32, tag="yt")
                nc.vector.tensor_scalar(
                    yt[:], ps0[ob][:], s4_t[:, ob:ob + 1], bi_t[:, ob:ob + 1],
                    mult, add)
                nc.sync.dma_start(yTd[ob * P:(ob + 1) * P, 0:TC], yt[:])

            # phase C: stream remaining tokens; alternate the two 4-bank
            # PSUM rings so consecutive chunks never wait on evictions.
            for c in range(1, nTC):
                t0 = c * TC
                xts = []
                for it in range(nI):
                    xt = xpool.tile([P, TC], F16, tag=f"x{it}", name=f"x{it}")
                    nc.scalar.dma_start(
                        xt[:], xTd[it * P:(it + 1) * P, t0:t0 + TC])
                    xts.append(xt)
                tag = "ps0" if c % 2 else "ps"
                pso = [pspool.tile([P, TC], F32, tag=tag, name="pso", bufs=4)
                       for _ in range(nOB)]
                for it in range(nI):
                    for ob in range(nOB):
                        nc.tensor.matmul(pso[ob][:],
                                         mt[it][:, ob * P:(ob + 1) * P],
                                         xts[it][:], start=(it == 0),
                                         stop=(it == nI - 1))
                for ob in range(nOB):
                    yt = ypool.tile([P, TC], F32, tag="yt")
                    nc.vector.tensor_scalar(
                        yt[:], pso[ob][:], s4_t[:, ob:ob + 1], bi_t[:, ob:ob + 1],
                        mult, add)
                    nc.sync.dma_start(yTd[ob * P:(ob + 1) * P, t0:t0 + TC], yt[:])

    nc.compile()
    return nc


_NC_CACHE = None


def _get_nc():
    global _NC_CACHE
    if _NC_CACHE is None:
        _NC_CACHE = _build_nc()
    return _NC_CACHE


def _col_major(v):
    return np.ascontiguousarray(
        np.asarray(v, dtype=np.float32).reshape(-1, P).T)


def make_in_maps(x, scaling0, B, scaling2, A, scaling4, bias):
    xh = np.asarray(x, dtype=np.float32).reshape(T_ALL, IN_D).astype(np.float16)
    xT = np.ascontiguousarray(xh.T)
    B16 = np.asarray(B, dtype=np.float32).astype(np.float16)
    AT = np.asarray(A, dtype=np.float32).T.astype(np.float16)
    s0c = _col_major(scaling0)
    s2c = _col_major(scaling2)

    in_maps = []
    for c in range(N_CORES):
        sh = slice(c * OS, (c + 1) * OS)
        sc = np.ascontiguousarray(np.concatenate(
            [s0c, s2c, _col_major(np.asarray(scaling4)[sh]),
             _col_major(np.asarray(bias)[sh])], axis=1))
        in_maps.append({
            "xT": xT, "B": B16,
            "ATs": np.ascontiguousarray(AT[:, sh]),
            "sc": sc,
        })
    return in_maps


def _unshard(results):
    y = np.empty((T_ALL, OUT_D), dtype=np.float32)
    for c in range(N_CORES):
        y[:, c * OS:(c + 1) * OS] = results[c]["yT"].T
    return y.reshape(BATCH, SEQ, OUT_D)


def kernel(x, scaling0, B, scaling2, A, scaling4, bias):
    # The profile hook isn't available in every environment; force the
    # plain execution path.
    os.environ.setdefault("BASS_NEVER_TRACE", "1")

    in_maps = make_in_maps(x, scaling0, B, scaling2, A, scaling4, bias)
    nc = _get_nc()
    res = bass_utils.run_bass_kernel_spmd(
        nc, in_maps, core_ids=list(range(N_CORES)))
    return _unshard(res.results)


# revision 6
# speedup vs baseline: 1.3731x; 1.0091x over previous
"""Trainium2 Bass kernel for DoubleBinaryLinear:
    y = ((x * s0) @ B.T * s2) @ A.T * s4 + bias
with x [4, 2048, 4096] fp32 and binary (+-1) B, A [4096, 4096].

v5: fused-weight restructure, output-sharded across the 8 cores.

    M.T = diag(s0) B.T (diag(s2) A.T)   # [in, out], token-independent
    y   = x @ M.T * s4 + bias           # x pre-cast fp16 on host

Core c computes M.T columns for its 512 output rows (one 4096x4096x512
matmul, ~220 us) and keeps M.T resident in SBUF, then streams ALL 8192
tokens through a single fused matmul (~440 us). Out-sharding makes each
core's M shard exactly what its own x-pass needs -- no collective, and
the token loop has no per-tile activations (s0 rides the M eviction
scale). All matmuls single-pass fp16 (binary weights exact; M and x
rounded once, rel err ~4e-4 << 2e-2 gate).

Per-matmul operands must come from plain [128, 512] tiles: slicing a
wider (batched-DMA) tile gives the operand AP a partition stride larger
than its free extent, which drops TensorE off its fast path (measured
259 ns vs 216 ns per 512-row matmul). So DMAs here are one trigger per
[128, 512] tile; evictions run on the Vector engine so the Scalar
engine is a pure x-prefetch stream and Sync carries B/A/y.
"""

import os

import numpy as np
import ml_dtypes

import concourse.bacc as bacc
import concourse.mybir as mybir
from concourse import tile
from concourse import bass_utils

P = 128
F32 = mybir.dt.float32
F16 = mybir.dt.float16
F8 = mybir.dt.float8e4

IN_D = 4096
MID_D = 4096
OUT_D = 4096
BATCH = 4
SEQ = 2048
N_CORES = 8
T_ALL = BATCH * SEQ                 # 8192 tokens, every core sees all
OS = OUT_D // N_CORES               # 512 output rows per core
TC = 512                            # matmul moving free dim
nI = IN_D // P                      # 32 in tiles
nM = MID_D // P                     # 32 mid tiles
nOB = OS // P                       # 4 out blocks per core
nTC = T_ALL // TC                   # 16 token chunks
IG = 4                              # in-tiles per M-compute PSUM group


def _build_nc():
    nc = bacc.Bacc(None, target_bir_lowering=False)
    xTd = nc.dram_tensor("xT", [IN_D, T_ALL], F16, kind="ExternalInput")
    Bd = nc.dram_tensor("B", [MID_D, IN_D], F16, kind="ExternalInput")
    ATd = nc.dram_tensor("ATs", [MID_D, OS], F8, kind="ExternalInput")
    nSC = nI + nM + 2 * nOB
    scd = nc.dram_tensor("sc", [P, nSC], F32, kind="ExternalInput")
    yTd = nc.dram_tensor("yT", [OS, T_ALL], F32, kind="ExternalOutput")

    mult = mybir.AluOpType.mult
    add = mybir.AluOpType.add

    with tile.TileContext(nc) as tc:
        with (
            tc.tile_pool(name="consts", bufs=1) as cpool,
            tc.tile_pool(name="a2buf", bufs=1) as apool,
            tc.tile_pool(name="mtbuf", bufs=1) as mpool,
            tc.tile_pool(name="xin", bufs=2) as xpool,
            tc.tile_pool(name="bwts", bufs=12) as bpool,
            tc.tile_pool(name="stage", bufs=4) as spool,
            tc.tile_pool(name="yout", bufs=6) as ypool,
            tc.tile_pool(name="psum", bufs=8, space="PSUM") as pspool,
        ):
            sc_t = cpool.tile([P, nSC], F32, tag="sc")
            nc.scalar.dma_start(sc_t[:], scd[:, :])
            s0_t = sc_t[:, 0:nI]
            s2_t = sc_t[:, nI:nI + nM]
            s4_t = sc_t[:, nI + nM:nI + nM + nOB]
            bi_t = sc_t[:, nI + nM + nOB:nSC]

            # phase A: a2[mk] = fp16(s2 * A.T) tiles [128 mid, OS]
            a2 = []
            for mk in range(nM):
                at = spool.tile([P, OS], F8, tag="at")
                nc.scalar.dma_start(at[:], ATd[mk * P:(mk + 1) * P, :])
                a2t = apool.tile([P, OS], F16, tag=f"a{mk}", name=f"a{mk}")
                nc.vector.tensor_scalar_mul(a2t[:], at[:], s2_t[:, mk:mk + 1])
                a2.append(a2t)

            # Chunk-0 x tiles prefetch right behind the AT loads so the
            # interleaved chunk-0 matmuls below have data by ~40 us.
            x0 = []
            for it in range(nI):
                xt = xpool.tile([P, TC], F16, tag=f"x{it}", name=f"x{it}")
                nc.scalar.dma_start(xt[:], xTd[it * P:(it + 1) * P, 0:TC])
                x0.append(xt)
            # Chunk-0 accumulators held across all of phase B (4 banks);
            # phase B itself rings through the other 4.
            ps0 = [pspool.tile([P, TC], F32, tag="ps0", name="ps0", bufs=4)
                   for _ in range(nOB)]

            def x0_mm(j, ig):
                # j-th (0..15) interleave slot while phase-B group ig runs:
                # chunk-0 matmul for an in-tile of group ig-1.
                it = (ig - 1) * IG + j // nOB
                ob = j % nOB
                nc.tensor.matmul(ps0[ob][:], mt[it][:, ob * P:(ob + 1) * P],
                                 x0[it][:], start=(it == 0),
                                 stop=(it == nI - 1))

            # phase B: M.T tiles [128 in, OS]; s0 folded into eviction.
            # One chunk-0 matmul per two B-steps stretches the B-stream
            # demand timeline ~11% so DMA jitter stops stalling TensorE.
            mt = [mpool.tile([P, OS], F16, tag=f"m{it}", name=f"m{it}")
                  for it in range(nI)]
            for ig in range(nI // IG):
                psb = [pspool.tile([P, OS], F32, tag="ps", name="ps", bufs=4)
                       for _ in range(IG)]
                for mk in range(nM):
                    bt = bpool.tile([P, IG * P], F16, tag="wb")
                    nc.sync.dma_start(
                        bt[:], Bd[mk * P:(mk + 1) * P,
                                  ig * IG * P:(ig + 1) * IG * P])
                    last = mk == nM - 1
                    for t_ in range(IG):
                        nc.tensor.matmul(psb[t_][:], bt[:, t_ * P:(t_ + 1) * P],
                                         a2[mk][:], start=(mk == 0), stop=last)
                    if ig > 0 and mk % 2 == 1:
                        x0_mm(mk // 2, ig)
                for t_ in range(IG):
                    it = ig * IG + t_
                    nc.vector.tensor_scalar_mul(mt[it][:], psb[t_][:],
                                                s0_t[:, it:it + 1])
            for j in range(16):
                x0_mm(j, nI // IG)
            for ob in range(nOB):
                yt = ypool.tile([P, TC], F32, tag="yt")
                nc.vector.tensor_scalar(
                    yt[:], ps0[ob][:], s4_t[:, ob:ob + 1], bi_t[:, ob:ob + 1],
                    mult, add)
                nc.sync.dma_start(yTd[ob * P:(ob + 1) * P, 0:TC], yt[:])

            # phase C: stream remaining tokens; alternate the two 4-bank
            # PSUM rings so consecutive chunks never wait on evictions.
            for c in range(1, nTC):
                t0 = c * TC
                xts = []
                for it in range(nI):
                    xt = xpool.tile([P, TC], F16, tag=f"x{it}", name=f"x{it}")
                    nc.scalar.dma_start(
                        xt[:], xTd[it * P:(it + 1) * P, t0:t0 + TC])
                    xts.append(xt)
                tag = "ps0" if c % 2 else "ps"
                pso = [pspool.tile([P, TC], F32, tag=tag, name="pso", bufs=4)
                       for _ in range(nOB)]
                for it in range(nI):
                    for ob in range(nOB):
                        nc.tensor.matmul(pso[ob][:],
                                         mt[it][:, ob * P:(ob + 1) * P],
                                         xts[it][:], start=(it == 0),
                                         stop=(it == nI - 1))
                for ob in range(nOB):
                    yt = ypool.tile([P, TC], F32, tag="yt")
                    nc.vector.tensor_scalar(
                        yt[:], pso[ob][:], s4_t[:, ob:ob + 1], bi_t[:, ob:ob + 1],
                        mult, add)
                    nc.sync.dma_start(yTd[ob * P:(ob + 1) * P, t0:t0 + TC], yt[:])

    nc.compile()
    return nc


_NC_CACHE = None


def _get_nc():
    global _NC_CACHE
    if _NC_CACHE is None:
        _NC_CACHE = _build_nc()
    return _NC_CACHE


def _col_major(v):
    return np.ascontiguousarray(
        np.asarray(v, dtype=np.float32).reshape(-1, P).T)


def make_in_maps(x, scaling0, B, scaling2, A, scaling4, bias):
    xh = np.asarray(x, dtype=np.float32).reshape(T_ALL, IN_D).astype(np.float16)
    xT = np.ascontiguousarray(xh.T)
    B16 = np.asarray(B, dtype=np.float32).astype(np.float16)
    AT = np.asarray(A, dtype=np.float32).T.astype(ml_dtypes.float8_e4m3)
    s0c = _col_major(scaling0)
    s2c = _col_major(scaling2)

    in_maps = []
    for c in range(N_CORES):
        sh = slice(c * OS, (c + 1) * OS)
        sc = np.ascontiguousarray(np.concatenate(
            [s0c, s2c, _col_major(np.asarray(scaling4)[sh]),
             _col_major(np.asarray(bias)[sh])], axis=1))
        in_maps.append({
            "xT": xT, "B": B16,
            "ATs": np.ascontiguousarray(AT[:, sh]),
            "sc": sc,
        })
    return in_maps


def _unshard(results):
    y = np.empty((T_ALL, OUT_D), dtype=np.float32)
    for c in range(N_CORES):
        y[:, c * OS:(c + 1) * OS] = results[c]["yT"].T
    return y.reshape(BATCH, SEQ, OUT_D)


def kernel(x, scaling0, B, scaling2, A, scaling4, bias):
    # The profile hook isn't available in every environment; force the
    # plain execution path.
    os.environ.setdefault("BASS_NEVER_TRACE", "1")

    in_maps = make_in_maps(x, scaling0, B, scaling2, A, scaling4, bias)
    nc = _get_nc()
    res = bass_utils.run_bass_kernel_spmd(
        nc, in_maps, core_ids=list(range(N_CORES)))
    return _unshard(res.results)
